# revision 39
# baseline (speedup 1.0000x reference)
"""ContextQueryAttention (BiDAF-style) Trainium2 kernel, 8-core data-parallel.

Math (per batch):
  s[i,j]  = wq.q_j + wc.c_i + sum_d c_id * wcq_d * q_jd          (L1 x L2)
  s1      = softmax_i(s * mq_j + (1-mq_j)*NEG)                   (softmax over i)
  s2      = softmax_i(s * mp_i + (1-mp_i)*NEG)
  a       = s1 @ Q                 (L1 x D)
  b       = (s1 @ s2^T) @ C  ==  s1 @ (s2^T @ C)   <- reassociated, no L1xL1
  out     = [C, a, C*a, C*b]                                      (L1 x 4D)

Kernel structure (v5 — bf16 datapath + lean scalar):
 - qwq_j is constant along the softmax axis (i) in both softmaxes -> cancels.
 - wc folded into the shared query-side operand qtw[d,j] = wcq_d*q[j,d] + wc_d
   so BOTH score matmuls produce  psum = dot + cwc_i  directly:
     E1 layout [j part, i free]: stationary qtw tile, moving ct
     E2 layout [i part, j free]: stationary ct tile,  moving qtw
 - mask_q applied via the e1 ACT's per-partition scale operand (j is the
   partition axis there): e1 = exp(mq_j * st). No broadcast mq tensor, no
   masked copy of qtw. Masked col -> exp(0)=1 -> uniform 1/L1; z1 fixed as
   z1 = mq_j*(colsum - L1) + L1 from the t-matmul's '1' column.
 - t matmul moving operand c1m[i,:] = [mp_i*c_i | mp_i | 1] gives t, z2 AND
   z1 raw column sums in one accumulation. No reduce ops.
 - whole datapath bf16 except PSUM accumulation: context loaded bf16, output
   tile + DRAM output bf16 (host widens to f32; tolerance is 2e-2, bf16
   roundoff ~2e-3). Halves both fat DMAs and DVE elem throughput cost.
 - scalar engine runs ONLY the 32 exp ACTs (the hard floor, ~(N+254)/1.2ns
   each); all DMA issues and copies live on sync/gpsimd/vector.
 - PE pre-warm via dummy matmuls on a memset tile (no make_identity dep, so
   warmup starts right after the framework preamble and keeps the HAM clock
   at 2.4GHz through the input-DMA window).
 - input DMAs: aux0+ct0 issued first so e2(b0) can start ~9us; qt/qnb packed
   as one tensor per batch; issue spread sync/gpsimd only.
 - PSUM accumulation groups never interleave within a bank (start=True clears
   has_written for the whole bank).
"""

import numpy as np

import concourse.bass as bass
import concourse.mybir as mybir
import concourse.tile as tile
from concourse import bacc
from concourse import bass_utils

F32 = mybir.dt.float32
BF16 = mybir.dt.bfloat16
EXP = mybir.ActivationFunctionType.Exp
ADD = mybir.AluOpType.add
MULT = mybir.AluOpType.mult

B, L1, L2, D = 16, 2048, 512, 128
NCORES = 8
BPC = B // NCORES          # batches per core
NT1 = L1 // 128            # 16 i-tiles
NT2 = L2 // 128            # 4  j-tiles
NWARM = 10                 # PE pre-warm dummy matmuls (512-wide)


def _build_program(dbg=False):
    nc = bacc.Bacc("TRN2", target_bir_lowering=False, debug=False)

    # aux[b] = [mp_t (16) | mq_t (4) | w^T (3)] as [128, 23] f32, host-packed
    aux_d = nc.dram_tensor("aux", [BPC, 128, NT1 + NT2 + 3], F32,
                           kind="ExternalInput").ap()
    # qpack[b] = [qt (4x128) | qnb (4x128)] bf16, host-packed
    qpack_d = nc.dram_tensor("qpack", [BPC, 128, 2 * NT2 * 128], BF16,
                             kind="ExternalInput").ap()
    ct_d = nc.dram_tensor("ct", [BPC, 128, NT1, 128], BF16,
                          kind="ExternalInput").ap()
    c1b_d = nc.dram_tensor("c1b", [BPC, 128, NT1, 128], BF16,
                           kind="ExternalInput").ap()
    # device output = [a | c*a | c*b]; the context passthrough section is
    # assembled host-side from the (exact f32) input during unshard
    out_d = nc.dram_tensor("out", [BPC, L1, 3 * D], BF16, kind="ExternalOutput").ap()

    with tile.TileContext(nc) as tc:
        with (
            tc.tile_pool(name="const", bufs=1) as const,
            tc.tile_pool(name="big", bufs=2) as big,
            tc.tile_pool(name="work", bufs=2) as work,
            tc.tile_pool(name="outp", bufs=3) as outp,
            tc.tile_pool(name="strips", bufs=2, space="PSUM") as strips,
            tc.tile_pool(name="small", bufs=2, space="PSUM") as small,
        ):
            # PE pre-warm + gap-fill: dummy matmuls on a memset tile. The PE
            # clock gates down on ANY idle gap and takes ~3us of continuous
            # execution to return to 2.4GHz, so warm mms pad the input-DMA
            # window and known dependency waits (ab(1) tail) to keep the
            # busy-streak alive. They write into a THROWAWAY tile from the
            # pool the next real accumulation uses: its first start=True
            # clears the garbage, so no dedicated PSUM bank is needed.
            warm_b = const.tile([128, 512], BF16)
            nc.gpsimd.memset(warm_b, 0.125)

            def warm_fill(pool, shape, n, w):
                wt = pool.tile(shape, F32, tag=("strip" if pool is strips
                                                else "acc"), name="warmt")
                for _ in range(n):
                    nc.tensor.matmul(
                        wt.rearrange("p a b -> p (a b)")[:, 0:w],
                        warm_b[:, 0:128], warm_b[:, 0:w],
                        start=True, stop=True,
                    )

            warm_fill(strips, [128, 2, 512], NWARM, 512)

            S = [dict() for _ in range(BPC)]  # per-batch tile state

            def ph_dma_in():
                # per-queue DMA bandwidth is limited (~100-160GB/s), so the
                # early-critical tensors are small SEPARATE TILES (dependency
                # tracking is tile-granular) spread across the three issuing
                # queues:
                #   sync:   aux0, ct0a, qnb0, ct1
                #   scalar: qt0, ct0b, qpack1     (done by ~7.6us, then ACTs)
                #   gpsimd: c1b0, aux1, c1b1
                for b in range(BPC):
                    s = S[b]
                    s["aux"] = work.tile([128, NT1 + NT2 + 3], F32, tag="aux",
                                         name=f"aux{b}")
                    s["mp"] = s["aux"][:, 0:NT1]
                    s["mq"] = s["aux"][:, NT1 : NT1 + NT2]
                    s["wc"] = s["aux"][:, NT1 + NT2 + 1 : NT1 + NT2 + 2]
                    s["wcq"] = s["aux"][:, NT1 + NT2 + 2 : NT1 + NT2 + 3]
                    s["c1b"] = big.tile([128, NT1, 128], BF16, tag="c1b",
                                        name=f"c1b{b}")
                s0, s1 = S[0], S[1]
                q4 = NT1 // 4
                s0["ct_a"] = big.tile([128, q4, 128], BF16, tag="ct_a",
                                      name="ct0a")
                s0["ct_b"] = big.tile([128, q4, 128], BF16, tag="ct_b",
                                      name="ct0b")
                s0["ct_c"] = big.tile([128, 2 * q4, 128], BF16, tag="ct_c",
                                      name="ct0c")
                s0["qt"] = work.tile([128, L2], BF16, tag="qt", name="qt0")
                s0["qnb"] = work.tile([128, L2], BF16, tag="qnb", name="qnb0")
                s1["qpack"] = work.tile([128, 2 * L2], BF16, tag="qpack",
                                        name="qpack1")
                s1["ct"] = big.tile([128, NT1, 128], BF16, tag="ct",
                                    name="ct1")
                s1["qt"] = s1["qpack"][:, 0:L2]
                s1["qnb"] = s1["qpack"][:, L2 : 2 * L2]
                nc.sync.dma_start(out=s0["aux"], in_=aux_d[0])
                nc.scalar.dma_start(out=s0["qt"], in_=qpack_d[0][:, 0:L2])
                nc.sync.dma_start(out=s0["ct_a"], in_=ct_d[0][:, 0:q4])
                nc.scalar.dma_start(out=s0["ct_b"], in_=ct_d[0][:, q4 : 2 * q4])
                nc.gpsimd.dma_start(out=s0["ct_c"], in_=ct_d[0][:, 2 * q4 : NT1])
                nc.sync.dma_start(out=s0["qnb"], in_=qpack_d[0][:, L2 : 2 * L2])
                nc.gpsimd.dma_start(out=s0["c1b"], in_=c1b_d[0])
                nc.scalar.dma_start(out=s1["qpack"], in_=qpack_d[1])
                nc.gpsimd.dma_start(out=s1["aux"], in_=aux_d[1])
                nc.sync.dma_start(out=s1["ct"], in_=ct_d[1])
                nc.gpsimd.dma_start(out=s1["c1b"], in_=c1b_d[1])

            def ct_sl(b, it0, n):
                # contiguous it-range [it0, it0+n) of batch b's transposed ct
                s = S[b]
                if b == 1:
                    return s["ct"][:, it0 : it0 + n]
                q4 = NT1 // 4
                if it0 + n <= q4:
                    return s["ct_a"][:, it0 : it0 + n]
                if it0 >= q4 and it0 + n <= 2 * q4:
                    return s["ct_b"][:, it0 - q4 : it0 - q4 + n]
                assert it0 >= 2 * q4
                return s["ct_c"][:, it0 - 2 * q4 : it0 - 2 * q4 + n]

            def ph_qprep(b):
                s = S[b]
                # qtw = qt*wcq + wc  (shared operand: e2 moving, e1 stationary)
                qtw = work.tile([128, L2], BF16, tag="qtw", name=f"qtw{b}")
                nc.vector.tensor_scalar(
                    out=qtw, in0=s["qt"], scalar1=s["wcq"],
                    scalar2=s["wc"], op0=MULT, op1=ADD,
                )
                s["qtw"] = qtw

            def ph_mask(b):
                # c1m = [mp_i * c | mp_i | 1]  (mask_p + z2 + z1-colsum operand)
                s = S[b]
                c1m = big.tile([128, NT1, 130], BF16, tag="c1m")
                for it in range(NT1):
                    nc.vector.tensor_scalar_mul(
                        c1m[:, it, 0:128], s["c1b"][:, it, :], s["mp"][:, it : it + 1]
                    )
                nc.gpsimd.tensor_copy(
                    c1m[:, :, 128:129].rearrange("p a b -> p (a b)"), s["mp"]
                )
                nc.gpsimd.memset(c1m[:, :, 129:130], 1.0)
                s["c1m"] = c1m

            def e2_strip(b, g):
                # e2[i,j] = exp(dot + cwc_i), unmasked (mp applied via c1m)
                s = S[b]
                if "e2n" not in s:
                    s["e2n"] = big.tile([128, NT1, L2], BF16, tag="e2n",
                                        name=f"e2n{b}")
                st = strips.tile([128, 2, 512], F32, tag="strip")
                for k in range(2):
                    ctt = ct_sl(b, 2 * g + k, 1)
                    nc.tensor.matmul(
                        st[:, k, :], ctt[:, 0, :], s["qtw"],
                        start=True, stop=True,
                    )
                nc.scalar.activation(s["e2n"][:, 2 * g : 2 * g + 2, :], st, EXP)

            def e1_strip(b, jt, h):
                # e1[j,i] = exp(mq_j * (dot + cwc_i)); mq via per-partition
                # ACT scale (j on partitions). masked col -> 1 (uniform)
                s = S[b]
                if "e1" not in s:
                    s["e1"] = big.tile([128, NT2, L1], BF16, tag="e1",
                                       name=f"e1_{b}")
                st = strips.tile([128, 2, 512], F32, tag="strip")
                for k in range(2):
                    m = 2 * h + k
                    nc.tensor.matmul(
                        st[:, k, :], s["qtw"][:, jt * 128 : (jt + 1) * 128],
                        ct_sl(b, 4 * m, 4),
                        start=True, stop=True,
                    )
                nc.scalar.activation(
                    s["e1"][:, jt, 1024 * h : 1024 * (h + 1)],
                    st.rearrange("p a b -> p (a b)"), EXP,
                    scale=s["mq"][:, jt : jt + 1],
                )

            def t_pair(b, jt, g):
                # partial t accumulation (it = 2g, 2g+1): only needs e2n's
                # ACT #g, so these interleave into the e1 chunk stream and
                # fill its strip-rotation waits with real work
                s = S[b]
                if "pst" not in s:
                    s["pst"] = {}
                if jt not in s["pst"]:
                    s["pst"][jt] = small.tile([128, 130], F32, tag="acc",
                                              name=f"pst{b}_{jt}")
                pst = s["pst"][jt]
                for k in range(2):
                    it = 2 * g + k
                    nc.tensor.matmul(
                        pst, s["e2n"][:, it, jt * 128 : (jt + 1) * 128],
                        s["c1m"][:, it, :],
                        start=(it == 0), stop=(it == NT1 - 1),
                    )

            def t_fin(b, jt):
                s = S[b]
                if "rhs_ab" not in s:
                    s["rhs_ab"] = work.tile([128, NT2, 256], BF16, tag="rhs_ab",
                                            name=f"rhs{b}")
                rhs_ab = s["rhs_ab"]
                pst = s["pst"][jt]
                # z1_j = mq_j*(colsum_j - L1) + L1 ; colsum in pst[:,129]
                z1 = work.tile([128, 1], F32, tag="z1")
                nc.vector.scalar_tensor_tensor(
                    out=z1, in0=pst[:, 129:130], scalar=-float(L1),
                    in1=s["mq"][:, jt : jt + 1], op0=ADD, op1=MULT,
                )
                nc.vector.tensor_scalar_add(z1, z1, float(L1))
                rz1 = work.tile([128, 1], F32, tag="rz1")
                nc.vector.reciprocal(rz1, z1)
                rz2 = work.tile([128, 1], F32, tag="rz2")
                nc.vector.reciprocal(rz2, pst[:, 128:129])
                rz12 = work.tile([128, 1], F32, tag="rz12")
                nc.vector.tensor_mul(rz12, rz2, rz1)
                nc.vector.tensor_scalar_mul(
                    rhs_ab[:, jt, 128:256], pst[:, 0:128], rz12
                )
                nc.vector.tensor_scalar_mul(
                    rhs_ab[:, jt, 0:128], s["qnb"][:, jt * 128 : (jt + 1) * 128],
                    rz1,
                )

            def ab_group(b, it0, tg, eng, fill=0, stagger=0):
                s = S[b]
                # fill/stagger: warm mms into a sacrificial acc tile (its
                # banks get reclaimed by the NEXT group's start=True) pad the
                # waits for this group's gating exp ACTs so its matmuls run
                # on a hot PE. Only legal while no OTHER accumulation shares
                # those banks.
                wt = None
                if fill or stagger:
                    wt = small.tile([128, 4, 256], F32, tag="acc", name="warmt")

                def wfill(n):
                    for _ in range(n):
                        nc.tensor.matmul(
                            wt[:, 0, :], warm_b[:, 0:128], warm_b[:, 0:256],
                            start=True, stop=True,
                        )

                wfill(fill)
                psab = small.tile([128, tg, 256], F32, tag="acc", name="psab")
                # groups must accumulate consecutively: a start=True into a
                # bank clears has_written for the whole bank
                for gi in range(tg):
                    it = it0 + gi
                    for jt in range(NT2):
                        nc.tensor.matmul(
                            psab[:, gi, :],
                            s["e1"][:, jt, it * 128 : (it + 1) * 128],
                            s["rhs_ab"][:, jt, :],
                            start=(jt == 0), stop=(jt == NT2 - 1),
                        )
                        if stagger and gi == 0 and jt == NT2 - 2:
                            wfill(stagger)
                o_full = outp.tile([128, 4, 384], BF16, tag="o_sb", name="o_sb")
                o_sb = o_full[:, 0:tg]
                c_sl = s["c1b"][:, it0 : it0 + tg, :]
                # post-exp groups' a-copy rides the scalar engine (idle after
                # its last exp ACT); earlier groups stay on vector
                if b == 1 and it0 >= 8:
                    nc.scalar.copy(o_sb[:, :, 0:128], psab[:, :, 0:128])
                else:
                    nc.vector.tensor_copy(o_sb[:, :, 0:128], psab[:, :, 0:128])
                nc.vector.tensor_tensor(
                    o_sb[:, :, 128:256], c_sl, psab[:, :, 0:128], MULT
                )
                nc.vector.tensor_tensor(
                    o_sb[:, :, 256:384], c_sl, psab[:, :, 128:256], MULT
                )
                odst = out_d[b].rearrange("(p t) m -> p t m", t=NT1)
                if eng is None:
                    # final group: split across two queues for a faster drain
                    nc.sync.dma_start(
                        out=odst[:, it0 : it0 + 2, :], in_=o_sb[:, 0:2]
                    )
                    nc.gpsimd.dma_start(
                        out=odst[:, it0 + 2 : it0 + tg, :], in_=o_sb[:, 2:tg]
                    )
                else:
                    eng.dma_start(out=odst[:, it0 : it0 + tg, :], in_=o_sb)

            def ph_dbg(b):
                if not (dbg and b == 0):
                    return
                s = S[b]
                for name, key in [
                    ("dbg_e1", "e1"), ("dbg_e2n", "e2n"),
                    ("dbg_rhs_ab", "rhs_ab"), ("dbg_qtw", "qtw"),
                    ("dbg_c1m", "c1m"),
                ]:
                    src = s[key]
                    dd = nc.dram_tensor(
                        name, list(src.shape), src.dtype, kind="ExternalOutput"
                    ).ap()
                    nc.sync.dma_start(out=dd, in_=src)

            # v4-style phase alternation (best measured): each engine
            # always has independent cross-batch work adjacent in its queue.
            def e1_phase(b, weave_ab=False):
                # h-major e1 chunks. The t partials (whose e2n ACT deps are
                # already satisfied) fill the h=0 chunks' strip-rotation
                # waits, one jt per chunk, so every rhs_ab is final before
                # the h=1 half. For the terminal batch, the ab groups weave
                # into the h=1 chunk stream: their mms' ACT deps stagger in
                # while the strips recycle, and their o_sb vector work
                # overlaps the remaining exp ACTs instead of trailing them.
                for jt in range(NT2):
                    e1_strip(b, jt, 0)
                    for g in range(8):
                        t_pair(b, jt, g)
                    t_fin(b, jt)
                for k in range(NT2):
                    e1_strip(b, k, 1)
                    if weave_ab and k >= 1:
                        ab_group(1, 4 * (k - 1), 4,
                                 nc.gpsimd if k % 2 else nc.sync)
                if weave_ab:
                    ab_group(1, 8, 4, nc.sync)
                    ab_group(1, 12, 4, None)

            ph_dma_in()
            ph_qprep(0); ph_qprep(1)
            for g in range(8):
                e2_strip(0, g)
            ph_mask(0)
            e1_phase(0)
            ph_mask(1)
            # ab(0) groups interleave with e2(1) strips: the strips are
            # ACT-cadence-limited (strip bufs recycle on ACT drain), so the
            # ab matmuls fill the tensor engine's wait slots
            e2_strip(1, 0); e2_strip(1, 1)
            ab_group(0, 0, 4, nc.sync)
            e2_strip(1, 2); e2_strip(1, 3)
            ab_group(0, 4, 4, nc.sync)
            e2_strip(1, 4); e2_strip(1, 5)
            ab_group(0, 8, 4, nc.sync)
            e2_strip(1, 6); e2_strip(1, 7)
            ab_group(0, 12, 4, nc.sync)
            e1_phase(1, weave_ab=True)
            ph_dbg(0)

    nc.compile()
    return nc


_NC = None


def _get_nc():
    global _NC
    if _NC is None:
        _NC = _build_program()
    return _NC


def _make_in_maps(inputs):
    import ml_dtypes
    bf16 = ml_dtypes.bfloat16

    context, query, w = inputs["context"], inputs["query"], inputs["w"]
    w2 = np.asarray(w).reshape(3, D).astype(np.float32)  # rows: wq, wc, wcq
    mp = np.asarray(inputs["mask_p"]).astype(np.float32)  # (B, L1)
    mq = np.asarray(inputs["mask_q"]).astype(np.float32)  # (B, L2)
    # aux[b] = [mp_t (16) | mq_t (4) | w^T (3)] as [128, 23]
    mp_t = mp.reshape(B, 128, NT1)                        # (B, 128, 16)
    mq_t = mq.reshape(B, 128, NT2)                        # (B, 128, 4)
    wt = np.broadcast_to(w2.T[None], (B, 128, 3))         # (B, 128, 3)
    aux = np.ascontiguousarray(
        np.concatenate([mp_t, mq_t, wt], axis=2), dtype=np.float32
    )
    ctx = np.asarray(context, dtype=np.float32)
    qry = np.asarray(query, dtype=np.float32)
    ct_t = np.ascontiguousarray(
        ctx.reshape(B, 128, NT1, D).transpose(0, 3, 2, 1)
    ).astype(bf16)  # [B, d, it, p]
    c1b = ctx.reshape(B, 128, NT1, D).astype(bf16)        # [B, p, it, d]
    qt_t = np.ascontiguousarray(
        qry.reshape(B, 128, NT2, D).transpose(0, 3, 2, 1)
    ).astype(bf16)  # [B, d, jt, p]
    qnb_t = qry.reshape(B, 128, NT2, D).astype(bf16)      # [B, p, jt, d]
    qpack = np.ascontiguousarray(
        np.concatenate(
            [qt_t.reshape(B, 128, L2), qnb_t.reshape(B, 128, L2)], axis=2
        )
    )  # [B, 128, 1024] bf16
    in_maps = []
    for c in range(NCORES):
        sl = slice(c * BPC, (c + 1) * BPC)
        in_maps.append(
            {
                "aux": aux[sl],
                "qpack": qpack[sl],
                "ct": ct_t[sl],
                "c1b": c1b[sl],
            }
        )
    return in_maps


def kernel(context, query, w, mask_p, mask_q):
    nc = _get_nc()
    in_maps = _make_in_maps(
        {"context": context, "query": query, "w": w, "mask_p": mask_p, "mask_q": mask_q}
    )
    res = bass_utils.run_bass_kernel_spmd(nc, in_maps, core_ids=list(range(NCORES)))
    out = np.empty((B, L1, 4 * D), dtype=np.float32)
    out[:, :, 0:D] = np.asarray(context, dtype=np.float32)  # exact passthrough
    acb = np.concatenate([res.results[c]["out"] for c in range(NCORES)], axis=0)
    out[:, :, D:] = acb.astype(np.float32)
    return out


# revision 42
# speedup vs baseline: 1.2131x; 1.2131x over previous
"""ContextQueryAttention (BiDAF-style) Trainium2 kernel, 8-core data-parallel.

Math (per batch):
  s[i,j]  = wq.q_j + wc.c_i + sum_d c_id * wcq_d * q_jd          (L1 x L2)
  s1      = softmax_i(s * mq_j + (1-mq_j)*NEG)                   (softmax over i)
  s2      = softmax_i(s * mp_i + (1-mp_i)*NEG)
  a       = s1 @ Q                 (L1 x D)
  b       = (s1 @ s2^T) @ C  ==  s1 @ (s2^T @ C)   <- reassociated, no L1xL1
  out     = [C, a, C*a, C*b]                                      (L1 x 4D)

Kernel structure (v5 — bf16 datapath + lean scalar):
 - qwq_j is constant along the softmax axis (i) in both softmaxes -> cancels.
 - wc folded into the shared query-side operand qtw[d,j] = wcq_d*q[j,d] + wc_d
   so BOTH score matmuls produce  psum = dot + cwc_i  directly:
     E1 layout [j part, i free]: stationary qtw tile, moving ct
     E2 layout [i part, j free]: stationary ct tile,  moving qtw
 - mask_q applied via the e1 ACT's per-partition scale operand (j is the
   partition axis there): e1 = exp(mq_j * st). No broadcast mq tensor, no
   masked copy of qtw. Masked col -> exp(0)=1 -> uniform 1/L1; z1 fixed as
   z1 = mq_j*(colsum - L1) + L1 from the t-matmul's '1' column.
 - t matmul moving operand c1m[i,:] = [mp_i*c_i | mp_i | 1] gives t, z2 AND
   z1 raw column sums in one accumulation. No reduce ops.
 - whole datapath bf16 except PSUM accumulation: context loaded bf16, output
   tile + DRAM output bf16 (host widens to f32; tolerance is 2e-2, bf16
   roundoff ~2e-3). Halves both fat DMAs and DVE elem throughput cost.
 - scalar engine runs ONLY the 32 exp ACTs (the hard floor, ~(N+254)/1.2ns
   each); all DMA issues and copies live on sync/gpsimd/vector.
 - PE pre-warm via dummy matmuls on a memset tile (no make_identity dep, so
   warmup starts right after the framework preamble and keeps the HAM clock
   at 2.4GHz through the input-DMA window).
 - input DMAs: aux0+ct0 issued first so e2(b0) can start ~9us; qt/qnb packed
   as one tensor per batch; issue spread sync/gpsimd only.
 - PSUM accumulation groups never interleave within a bank (start=True clears
   has_written for the whole bank).
"""

import numpy as np

import concourse.bass as bass
import concourse.mybir as mybir
import concourse.tile as tile
from concourse import bacc
from concourse import bass_utils

F32 = mybir.dt.float32
BF16 = mybir.dt.bfloat16
EXP = mybir.ActivationFunctionType.Exp
ADD = mybir.AluOpType.add
MULT = mybir.AluOpType.mult

B, L1, L2, D = 16, 2048, 512, 128
NCORES = 8
BPC = B // NCORES          # batches per core
NT1 = L1 // 128            # 16 i-tiles
NT2 = L2 // 128            # 4  j-tiles
NWARM = 10                 # PE pre-warm dummy matmuls (512-wide)


def _build_program(dbg=False):
    nc = bacc.Bacc("TRN2", target_bir_lowering=False, debug=False)

    # aux[b] = [mp_t (16) | mq_t (4) | w^T (3)] as [128, 23] f32, host-packed
    aux_d = nc.dram_tensor("aux", [BPC, 128, NT1 + NT2 + 3], F32,
                           kind="ExternalInput").ap()
    # qpack[b] = [qt (4x128) | qnb (4x128)] bf16, host-packed
    qpack_d = nc.dram_tensor("qpack", [BPC, 128, 2 * NT2 * 128], BF16,
                             kind="ExternalInput").ap()
    ct_d = nc.dram_tensor("ct", [BPC, 128, NT1, 128], BF16,
                          kind="ExternalInput").ap()
    c1b_d = nc.dram_tensor("c1b", [BPC, 128, NT1, 128], BF16,
                           kind="ExternalInput").ap()
    # device output = [a | c*a | c*b]; the context passthrough section is
    # assembled host-side from the (exact f32) input during unshard
    out_d = nc.dram_tensor("out", [BPC, L1, 3 * D], BF16, kind="ExternalOutput").ap()

    with tile.TileContext(nc) as tc:
        with (
            tc.tile_pool(name="const", bufs=1) as const,
            tc.tile_pool(name="big", bufs=2) as big,
            tc.tile_pool(name="work", bufs=2) as work,
            tc.tile_pool(name="outp", bufs=3) as outp,
            tc.tile_pool(name="strips", bufs=2, space="PSUM") as strips,
            tc.tile_pool(name="small", bufs=2, space="PSUM") as small,
        ):
            # PE pre-warm + gap-fill: dummy matmuls on a memset tile. The PE
            # clock gates down on ANY idle gap and takes ~3us of continuous
            # execution to return to 2.4GHz, so warm mms pad the input-DMA
            # window and known dependency waits (ab(1) tail) to keep the
            # busy-streak alive. They write into a THROWAWAY tile from the
            # pool the next real accumulation uses: its first start=True
            # clears the garbage, so no dedicated PSUM bank is needed.
            warm_b = const.tile([128, 512], BF16)
            nc.gpsimd.memset(warm_b, 0.125)

            def warm_fill(pool, shape, n, w):
                wt = pool.tile(shape, F32, tag=("strip" if pool is strips
                                                else "acc"), name="warmt")
                for _ in range(n):
                    nc.tensor.matmul(
                        wt.rearrange("p a b -> p (a b)")[:, 0:w],
                        warm_b[:, 0:128], warm_b[:, 0:w],
                        start=True, stop=True,
                    )

            warm_fill(strips, [128, 2, 512], NWARM, 512)

            S = [dict() for _ in range(BPC)]  # per-batch tile state

            def ph_dma_in():
                # per-queue DMA bandwidth is limited (~100-160GB/s), so the
                # early-critical tensors are small SEPARATE TILES (dependency
                # tracking is tile-granular) spread across the three issuing
                # queues:
                #   sync:   aux0, ct0a, qnb0, ct1
                #   scalar: qt0, ct0b, qpack1     (done by ~7.6us, then ACTs)
                #   gpsimd: c1b0, aux1, c1b1
                for b in range(BPC):
                    s = S[b]
                    s["aux"] = work.tile([128, NT1 + NT2 + 3], F32, tag="aux",
                                         name=f"aux{b}")
                    s["mp"] = s["aux"][:, 0:NT1]
                    s["mq"] = s["aux"][:, NT1 : NT1 + NT2]
                    s["wc"] = s["aux"][:, NT1 + NT2 + 1 : NT1 + NT2 + 2]
                    s["wcq"] = s["aux"][:, NT1 + NT2 + 2 : NT1 + NT2 + 3]
                    s["c1b"] = big.tile([128, NT1, 128], BF16, tag="c1b",
                                        name=f"c1b{b}")
                s0, s1 = S[0], S[1]
                q4 = NT1 // 4
                s0["ct_a"] = big.tile([128, q4, 128], BF16, tag="ct_a",
                                      name="ct0a")
                s0["ct_b"] = big.tile([128, q4, 128], BF16, tag="ct_b",
                                      name="ct0b")
                s0["ct_c"] = big.tile([128, 2 * q4, 128], BF16, tag="ct_c",
                                      name="ct0c")
                s0["qt"] = work.tile([128, L2], BF16, tag="qt", name="qt0")
                s0["qnb"] = work.tile([128, L2], BF16, tag="qnb", name="qnb0")
                s1["qpack"] = work.tile([128, 2 * L2], BF16, tag="qpack",
                                        name="qpack1")
                s1["ct"] = big.tile([128, NT1, 128], BF16, tag="ct",
                                    name="ct1")
                s1["qt"] = s1["qpack"][:, 0:L2]
                s1["qnb"] = s1["qpack"][:, L2 : 2 * L2]
                nc.sync.dma_start(out=s0["aux"], in_=aux_d[0])
                nc.scalar.dma_start(out=s0["qt"], in_=qpack_d[0][:, 0:L2])
                nc.sync.dma_start(out=s0["ct_a"], in_=ct_d[0][:, 0:q4])
                nc.sync.dma_start(out=s0["ct_b"], in_=ct_d[0][:, q4 : 2 * q4])
                nc.gpsimd.dma_start(out=s0["ct_c"], in_=ct_d[0][:, 2 * q4 : NT1])
                nc.scalar.dma_start(out=s0["qnb"], in_=qpack_d[0][:, L2 : 2 * L2])
                nc.gpsimd.dma_start(out=s0["c1b"], in_=c1b_d[0])
                nc.scalar.dma_start(out=s1["qpack"], in_=qpack_d[1])
                nc.gpsimd.dma_start(out=s1["aux"], in_=aux_d[1])
                nc.sync.dma_start(out=s1["ct"], in_=ct_d[1])
                nc.gpsimd.dma_start(out=s1["c1b"], in_=c1b_d[1])

            def ct_sl(b, it0, n):
                # contiguous it-range [it0, it0+n) of batch b's transposed ct
                s = S[b]
                if b == 1:
                    return s["ct"][:, it0 : it0 + n]
                q4 = NT1 // 4
                if it0 + n <= q4:
                    return s["ct_a"][:, it0 : it0 + n]
                if it0 >= q4 and it0 + n <= 2 * q4:
                    return s["ct_b"][:, it0 - q4 : it0 - q4 + n]
                assert it0 >= 2 * q4
                return s["ct_c"][:, it0 - 2 * q4 : it0 - 2 * q4 + n]

            def ph_qprep(b):
                s = S[b]
                # qtw = qt*wcq + wc  (shared operand: e2 moving, e1 stationary)
                qtw = work.tile([128, L2], BF16, tag="qtw", name=f"qtw{b}")
                nc.vector.tensor_scalar(
                    out=qtw, in0=s["qt"], scalar1=s["wcq"],
                    scalar2=s["wc"], op0=MULT, op1=ADD,
                )
                s["qtw"] = qtw

            def ph_mask(b):
                # c1m = [mp_i * c | mp_i | 1]  (mask_p + z2 + z1-colsum operand)
                s = S[b]
                c1m = big.tile([128, NT1, 130], BF16, tag="c1m")
                for it in range(NT1):
                    nc.vector.tensor_scalar_mul(
                        c1m[:, it, 0:128], s["c1b"][:, it, :], s["mp"][:, it : it + 1]
                    )
                nc.gpsimd.tensor_copy(
                    c1m[:, :, 128:129].rearrange("p a b -> p (a b)"), s["mp"]
                )
                nc.gpsimd.memset(c1m[:, :, 129:130], 1.0)
                s["c1m"] = c1m

            def e2_strip(b, g):
                # e2[i,j] = exp(dot + cwc_i), unmasked (mp applied via c1m)
                s = S[b]
                if "e2n" not in s:
                    s["e2n"] = big.tile([128, NT1, L2], BF16, tag="e2n",
                                        name=f"e2n{b}")
                st = strips.tile([128, 2, 512], F32, tag="strip")
                for k in range(2):
                    ctt = ct_sl(b, 2 * g + k, 1)
                    nc.tensor.matmul(
                        st[:, k, :], ctt[:, 0, :], s["qtw"],
                        start=True, stop=True,
                    )
                nc.scalar.activation(s["e2n"][:, 2 * g : 2 * g + 2, :], st, EXP)

            def e1_strip(b, jt, h):
                # e1[j,i] = exp(mq_j * (dot + cwc_i)); mq via per-partition
                # ACT scale (j on partitions). masked col -> 1 (uniform)
                s = S[b]
                if "e1" not in s:
                    s["e1"] = big.tile([128, NT2, L1], BF16, tag="e1",
                                       name=f"e1_{b}")
                st = strips.tile([128, 2, 512], F32, tag="strip")
                for k in range(2):
                    m = 2 * h + k
                    nc.tensor.matmul(
                        st[:, k, :], s["qtw"][:, jt * 128 : (jt + 1) * 128],
                        ct_sl(b, 4 * m, 4),
                        start=True, stop=True,
                    )
                nc.scalar.activation(
                    s["e1"][:, jt, 1024 * h : 1024 * (h + 1)],
                    st.rearrange("p a b -> p (a b)"), EXP,
                    scale=s["mq"][:, jt : jt + 1],
                )

            def t_pair(b, jt, g):
                # partial t accumulation (it = 2g, 2g+1): only needs e2n's
                # ACT #g, so these interleave into the e1 chunk stream and
                # fill its strip-rotation waits with real work
                s = S[b]
                if "pst" not in s:
                    s["pst"] = {}
                if jt not in s["pst"]:
                    s["pst"][jt] = small.tile([128, 130], F32, tag="acc",
                                              name=f"pst{b}_{jt}")
                pst = s["pst"][jt]
                for k in range(2):
                    it = 2 * g + k
                    nc.tensor.matmul(
                        pst, s["e2n"][:, it, jt * 128 : (jt + 1) * 128],
                        s["c1m"][:, it, :],
                        start=(it == 0), stop=(it == NT1 - 1),
                    )

            def t_fin(b, jt):
                s = S[b]
                if "rhs_ab" not in s:
                    s["rhs_ab"] = work.tile([128, NT2, 256], BF16, tag="rhs_ab",
                                            name=f"rhs{b}")
                rhs_ab = s["rhs_ab"]
                pst = s["pst"][jt]
                # z1_j = mq_j*(colsum_j - L1) + L1 ; colsum in pst[:,129]
                z1 = work.tile([128, 1], F32, tag="z1")
                nc.vector.scalar_tensor_tensor(
                    out=z1, in0=pst[:, 129:130], scalar=-float(L1),
                    in1=s["mq"][:, jt : jt + 1], op0=ADD, op1=MULT,
                )
                nc.vector.tensor_scalar_add(z1, z1, float(L1))
                rz1 = work.tile([128, 1], F32, tag="rz1")
                nc.vector.reciprocal(rz1, z1)
                rz2 = work.tile([128, 1], F32, tag="rz2")
                nc.vector.reciprocal(rz2, pst[:, 128:129])
                rz12 = work.tile([128, 1], F32, tag="rz12")
                nc.vector.tensor_mul(rz12, rz2, rz1)
                nc.vector.tensor_scalar_mul(
                    rhs_ab[:, jt, 128:256], pst[:, 0:128], rz12
                )
                nc.vector.tensor_scalar_mul(
                    rhs_ab[:, jt, 0:128], s["qnb"][:, jt * 128 : (jt + 1) * 128],
                    rz1,
                )

            def ab_group(b, it0, tg, eng, fill=0, stagger=0):
                s = S[b]
                # fill/stagger: warm mms into a sacrificial acc tile (its
                # banks get reclaimed by the NEXT group's start=True) pad the
                # waits for this group's gating exp ACTs so its matmuls run
                # on a hot PE. Only legal while no OTHER accumulation shares
                # those banks.
                wt = None
                if fill or stagger:
                    wt = small.tile([128, 4, 256], F32, tag="acc", name="warmt")

                def wfill(n):
                    for _ in range(n):
                        nc.tensor.matmul(
                            wt[:, 0, :], warm_b[:, 0:128], warm_b[:, 0:256],
                            start=True, stop=True,
                        )

                wfill(fill)
                psab = small.tile([128, tg, 256], F32, tag="acc", name="psab")
                # groups must accumulate consecutively: a start=True into a
                # bank clears has_written for the whole bank
                for gi in range(tg):
                    it = it0 + gi
                    for jt in range(NT2):
                        nc.tensor.matmul(
                            psab[:, gi, :],
                            s["e1"][:, jt, it * 128 : (it + 1) * 128],
                            s["rhs_ab"][:, jt, :],
                            start=(jt == 0), stop=(jt == NT2 - 1),
                        )
                        if stagger and gi == 0 and jt == NT2 - 2:
                            wfill(stagger)
                o_full = outp.tile([128, 4, 384], BF16, tag="o_sb", name="o_sb")
                o_sb = o_full[:, 0:tg]
                c_sl = s["c1b"][:, it0 : it0 + tg, :]
                # post-exp groups' a-copy rides the scalar engine (idle after
                # its last exp ACT); earlier groups stay on vector
                if b == 1 and it0 >= 8:
                    nc.scalar.copy(o_sb[:, :, 0:128], psab[:, :, 0:128])
                else:
                    nc.vector.tensor_copy(o_sb[:, :, 0:128], psab[:, :, 0:128])
                nc.vector.tensor_tensor(
                    o_sb[:, :, 128:256], c_sl, psab[:, :, 0:128], MULT
                )
                nc.vector.tensor_tensor(
                    o_sb[:, :, 256:384], c_sl, psab[:, :, 128:256], MULT
                )
                odst = out_d[b].rearrange("(p t) m -> p t m", t=NT1)
                if eng is None:
                    # final group: split across two queues for a faster drain
                    nc.sync.dma_start(
                        out=odst[:, it0 : it0 + 2, :], in_=o_sb[:, 0:2]
                    )
                    nc.gpsimd.dma_start(
                        out=odst[:, it0 + 2 : it0 + tg, :], in_=o_sb[:, 2:tg]
                    )
                else:
                    eng.dma_start(out=odst[:, it0 : it0 + tg, :], in_=o_sb)

            def ph_dbg(b):
                if not (dbg and b == 0):
                    return
                s = S[b]
                for name, key in [
                    ("dbg_e1", "e1"), ("dbg_e2n", "e2n"),
                    ("dbg_rhs_ab", "rhs_ab"), ("dbg_qtw", "qtw"),
                    ("dbg_c1m", "c1m"),
                ]:
                    src = s[key]
                    dd = nc.dram_tensor(
                        name, list(src.shape), src.dtype, kind="ExternalOutput"
                    ).ap()
                    nc.sync.dma_start(out=dd, in_=src)

            # v4-style phase alternation (best measured): each engine
            # always has independent cross-batch work adjacent in its queue.
            def e1_phase(b):
                # h-major e1 chunks with the t partial accumulations woven
                # in (their e2n ACT deps are already satisfied, so they fill
                # the strip-rotation waits); each jt's rhs_ab finalizes one
                # chunk after its last partial -> ab can start on the h=0
                # ACTs without waiting for a separate t phase. (Weaving the
                # ab GROUPS into the h=1 chunk stream was tried and badly
                # regressed: a 16-mm group stalls strip production ~3.7us,
                # starving the exp ACT stream.)
                chunks = [(jt, h) for h in range(2) for jt in range(NT2)]
                for k, (jt, h) in enumerate(chunks):
                    e1_strip(b, jt, h)
                    tj = k // 2
                    if k % 2 == 0:
                        for g in range(4):
                            t_pair(b, tj, g)
                    else:
                        for g in range(4, 8):
                            t_pair(b, tj, g)
                        t_fin(b, tj)

            ph_dma_in()
            ph_qprep(0); ph_qprep(1)
            for g in range(8):
                e2_strip(0, g)
            ph_mask(0)
            e1_phase(0)
            ph_mask(1)
            # ab(0) groups interleave with e2(1) strips: the strips are
            # ACT-cadence-limited (strip bufs recycle on ACT drain), so the
            # ab matmuls fill the tensor engine's wait slots
            e2_strip(1, 0); e2_strip(1, 1)
            ab_group(0, 0, 4, nc.sync)
            e2_strip(1, 2); e2_strip(1, 3)
            ab_group(0, 4, 4, nc.sync)
            e2_strip(1, 4); e2_strip(1, 5)
            ab_group(0, 8, 4, nc.sync)
            e2_strip(1, 6); e2_strip(1, 7)
            ab_group(0, 12, 4, nc.sync)
            e1_phase(1)
            # b1 drains on alternating queues so two queues empty in parallel
            ab_group(1, 0, 4, nc.gpsimd)
            ab_group(1, 4, 4, nc.sync)
            ab_group(1, 8, 4, nc.gpsimd)
            ab_group(1, 12, 4, None)
            ph_dbg(0)

    nc.compile()
    return nc


_NC = None


def _get_nc():
    global _NC
    if _NC is None:
        _NC = _build_program()
    return _NC


def _make_in_maps(inputs):
    import ml_dtypes
    bf16 = ml_dtypes.bfloat16

    context, query, w = inputs["context"], inputs["query"], inputs["w"]
    w2 = np.asarray(w).reshape(3, D).astype(np.float32)  # rows: wq, wc, wcq
    mp = np.asarray(inputs["mask_p"]).astype(np.float32)  # (B, L1)
    mq = np.asarray(inputs["mask_q"]).astype(np.float32)  # (B, L2)
    # aux[b] = [mp_t (16) | mq_t (4) | w^T (3)] as [128, 23]
    mp_t = mp.reshape(B, 128, NT1)                        # (B, 128, 16)
    mq_t = mq.reshape(B, 128, NT2)                        # (B, 128, 4)
    wt = np.broadcast_to(w2.T[None], (B, 128, 3))         # (B, 128, 3)
    aux = np.ascontiguousarray(
        np.concatenate([mp_t, mq_t, wt], axis=2), dtype=np.float32
    )
    ctx = np.asarray(context, dtype=np.float32)
    qry = np.asarray(query, dtype=np.float32)
    ct_t = np.ascontiguousarray(
        ctx.reshape(B, 128, NT1, D).transpose(0, 3, 2, 1)
    ).astype(bf16)  # [B, d, it, p]
    c1b = ctx.reshape(B, 128, NT1, D).astype(bf16)        # [B, p, it, d]
    qt_t = np.ascontiguousarray(
        qry.reshape(B, 128, NT2, D).transpose(0, 3, 2, 1)
    ).astype(bf16)  # [B, d, jt, p]
    qnb_t = qry.reshape(B, 128, NT2, D).astype(bf16)      # [B, p, jt, d]
    qpack = np.ascontiguousarray(
        np.concatenate(
            [qt_t.reshape(B, 128, L2), qnb_t.reshape(B, 128, L2)], axis=2
        )
    )  # [B, 128, 1024] bf16
    in_maps = []
    for c in range(NCORES):
        sl = slice(c * BPC, (c + 1) * BPC)
        in_maps.append(
            {
                "aux": aux[sl],
                "qpack": qpack[sl],
                "ct": ct_t[sl],
                "c1b": c1b[sl],
            }
        )
    return in_maps


def kernel(context, query, w, mask_p, mask_q):
    nc = _get_nc()
    in_maps = _make_in_maps(
        {"context": context, "query": query, "w": w, "mask_p": mask_p, "mask_q": mask_q}
    )
    res = bass_utils.run_bass_kernel_spmd(nc, in_maps, core_ids=list(range(NCORES)))
    out = np.empty((B, L1, 4 * D), dtype=np.float32)
    out[:, :, 0:D] = np.asarray(context, dtype=np.float32)  # exact passthrough
    acb = np.concatenate([res.results[c]["out"] for c in range(NCORES)], axis=0)
    out[:, :, D:] = acb.astype(np.float32)
    return out


# revision 44
# speedup vs baseline: 1.2142x; 1.0009x over previous
"""ContextQueryAttention (BiDAF-style) Trainium2 kernel, 8-core data-parallel.

Math (per batch):
  s[i,j]  = wq.q_j + wc.c_i + sum_d c_id * wcq_d * q_jd          (L1 x L2)
  s1      = softmax_i(s * mq_j + (1-mq_j)*NEG)                   (softmax over i)
  s2      = softmax_i(s * mp_i + (1-mp_i)*NEG)
  a       = s1 @ Q                 (L1 x D)
  b       = (s1 @ s2^T) @ C  ==  s1 @ (s2^T @ C)   <- reassociated, no L1xL1
  out     = [C, a, C*a, C*b]                                      (L1 x 4D)

Kernel structure (v11, 62.3us vs 84.1us staged baseline; err 1.1e-4):
 - qwq_j is constant along the softmax axis (i) in both softmaxes -> cancels.
 - wc folded into the shared query-side operand qtw[d,j] = wcq_d*q[j,d] + wc_d
   so BOTH score matmuls produce  psum = dot + cwc_i  directly:
     E1 layout [j part, i free]: stationary qtw tile, moving ct
     E2 layout [i part, j free]: stationary ct tile,  moving qtw
 - mask_q applied via the e1 ACT's per-partition scale operand (j is the
   partition axis there): e1 = exp(mq_j * st); costs +90ns/ACT but kills the
   broadcast-mq tensor + masked qtw copy. Masked col -> exp(0)=1 -> uniform
   1/L1; z1 fixed as z1 = mq_j*(colsum - L1) + L1 from the '1' column.
 - t matmul moving operand c1m[i,:] = [mp_i*c_i | mp_i | 1] gives t, z2 AND
   z1 raw column sums in one accumulation. No reduce ops.
 - whole datapath bf16 except PSUM accumulation; DRAM output bf16 and only
   [a|c*a|c*b] — the context passthrough section is assembled host-side in
   exact f32 (tolerance 2e-2, measured 1.1e-4). DMA 12.4MB -> 6.4MB.
 - scalar engine runs ONLY the 32 exp ACTs (the ~35us serial backbone;
   (N+~260)/1.2ns each, N=1024 bounded by 2-bank strips — widening needs
   PSUM that isn't there) plus the two post-exp a-copies.
 - the whole schedule serves the scalar exp stream: first ACT ~12us (DMA
   bound), zero mid-gaps, T_exp ~47.5us, tail ~14us.
 - t accumulations are split into it-pairs and woven into the e1 chunk
   stream (their e2n ACT deps are long satisfied) so rhs_ab is final before
   the h=1 half and ab needs no separate t phase; ab(0) groups interleave
   with e2(1) strips. Weaving ab GROUPS into e1(1)'s h=1 chunks regresses
   badly (16-mm groups stall strip production -> exp stream gaps).
 - PE clock model (cost_model): ANY idle gap drops the PE to 1.2GHz and it
   needs 3us of CONTINUOUS execution to return to 2.4GHz. Warm mms (memset
   operand, throwaway PSUM tile whose banks the next accumulation's
   start=True reclaims) cover the input-DMA window. Over-filling delays
   in-order work: warm fills between ab tail groups measured WORSE (the
   tail is paced by the psab double-buffer's slow readers, not the clock).
 - per-DMA-queue bandwidth is only ~85-130GB/s: inputs are split into small
   single-reader tiles (ct0 in quarters, qt/qnb separate) spread across the
   sync/scalar/gpsimd queues, ordered by first use; dependency tracking is
   tile-granular, so two DMAs into one tile both gate the first reader.
 - out DMAs: b0 groups on sync while b1 computes; b1 alternates
   gpsimd/sync; the final group splits across both queues.
 - PSUM accumulation groups never interleave within a bank (start=True
   clears has_written for the whole bank); pst(jt0)/pst(jt1) in different
   acc bufs may interleave their accumulations legally.
"""

import numpy as np

import concourse.bass as bass
import concourse.mybir as mybir
import concourse.tile as tile
from concourse import bacc
from concourse import bass_utils

F32 = mybir.dt.float32
BF16 = mybir.dt.bfloat16
EXP = mybir.ActivationFunctionType.Exp
ADD = mybir.AluOpType.add
MULT = mybir.AluOpType.mult

B, L1, L2, D = 16, 2048, 512, 128
NCORES = 8
BPC = B // NCORES          # batches per core
NT1 = L1 // 128            # 16 i-tiles
NT2 = L2 // 128            # 4  j-tiles
NWARM = 10                 # PE pre-warm dummy matmuls (512-wide)


def _build_program(dbg=False):
    nc = bacc.Bacc("TRN2", target_bir_lowering=False, debug=False)

    # aux[b] = [mp_t (16) | mq_t (4) | w^T (3)] as [128, 23] f32, host-packed
    aux_d = nc.dram_tensor("aux", [BPC, 128, NT1 + NT2 + 3], F32,
                           kind="ExternalInput").ap()
    # qpack[b] = [qt (4x128) | qnb (4x128)] bf16, host-packed
    qpack_d = nc.dram_tensor("qpack", [BPC, 128, 2 * NT2 * 128], BF16,
                             kind="ExternalInput").ap()
    ct_d = nc.dram_tensor("ct", [BPC, 128, NT1, 128], BF16,
                          kind="ExternalInput").ap()
    c1b_d = nc.dram_tensor("c1b", [BPC, 128, NT1, 128], BF16,
                           kind="ExternalInput").ap()
    # device output = [a | c*a | c*b]; the context passthrough section is
    # assembled host-side from the (exact f32) input during unshard
    out_d = nc.dram_tensor("out", [BPC, L1, 3 * D], BF16, kind="ExternalOutput").ap()

    with tile.TileContext(nc) as tc:
        with (
            tc.tile_pool(name="const", bufs=1) as const,
            tc.tile_pool(name="big", bufs=2) as big,
            tc.tile_pool(name="work", bufs=2) as work,
            tc.tile_pool(name="outp", bufs=3) as outp,
            tc.tile_pool(name="strips", bufs=2, space="PSUM") as strips,
            tc.tile_pool(name="small", bufs=2, space="PSUM") as small,
        ):
            # PE pre-warm + gap-fill: dummy matmuls on a memset tile. The PE
            # clock gates down on ANY idle gap and takes ~3us of continuous
            # execution to return to 2.4GHz, so warm mms pad the input-DMA
            # window and known dependency waits (ab(1) tail) to keep the
            # busy-streak alive. They write into a THROWAWAY tile from the
            # pool the next real accumulation uses: its first start=True
            # clears the garbage, so no dedicated PSUM bank is needed.
            warm_b = const.tile([128, 512], BF16)
            nc.gpsimd.memset(warm_b, 0.125)

            def warm_fill(pool, shape, n, w):
                wt = pool.tile(shape, F32, tag=("strip" if pool is strips
                                                else "acc"), name="warmt")
                for _ in range(n):
                    nc.tensor.matmul(
                        wt.rearrange("p a b -> p (a b)")[:, 0:w],
                        warm_b[:, 0:128], warm_b[:, 0:w],
                        start=True, stop=True,
                    )

            warm_fill(strips, [128, 2, 512], NWARM, 512)

            S = [dict() for _ in range(BPC)]  # per-batch tile state

            def ph_dma_in():
                # per-queue DMA bandwidth is limited (~100-160GB/s), so the
                # early-critical tensors are small SEPARATE TILES (dependency
                # tracking is tile-granular) spread across the three issuing
                # queues:
                #   sync:   aux0, ct0a, qnb0, ct1
                #   scalar: qt0, ct0b, qpack1     (done by ~7.6us, then ACTs)
                #   gpsimd: c1b0, aux1, c1b1
                for b in range(BPC):
                    s = S[b]
                    s["aux"] = work.tile([128, NT1 + NT2 + 3], F32, tag="aux",
                                         name=f"aux{b}")
                    s["mp"] = s["aux"][:, 0:NT1]
                    s["mq"] = s["aux"][:, NT1 : NT1 + NT2]
                    s["wc"] = s["aux"][:, NT1 + NT2 + 1 : NT1 + NT2 + 2]
                    s["wcq"] = s["aux"][:, NT1 + NT2 + 2 : NT1 + NT2 + 3]
                    s["c1b"] = big.tile([128, NT1, 128], BF16, tag="c1b",
                                        name=f"c1b{b}")
                s0, s1 = S[0], S[1]
                q4 = NT1 // 4
                s0["ct_a"] = big.tile([128, q4, 128], BF16, tag="ct_a",
                                      name="ct0a")
                s0["ct_b"] = big.tile([128, q4, 128], BF16, tag="ct_b",
                                      name="ct0b")
                s0["ct_c"] = big.tile([128, 2 * q4, 128], BF16, tag="ct_c",
                                      name="ct0c")
                s0["qt"] = work.tile([128, L2], BF16, tag="qt", name="qt0")
                s0["qnb"] = work.tile([128, L2], BF16, tag="qnb", name="qnb0")
                s1["qpack"] = work.tile([128, 2 * L2], BF16, tag="qpack",
                                        name="qpack1")
                s1["ct"] = big.tile([128, NT1, 128], BF16, tag="ct",
                                    name="ct1")
                s1["qt"] = s1["qpack"][:, 0:L2]
                s1["qnb"] = s1["qpack"][:, L2 : 2 * L2]
                nc.sync.dma_start(out=s0["aux"], in_=aux_d[0])
                nc.scalar.dma_start(out=s0["qt"], in_=qpack_d[0][:, 0:L2])
                nc.sync.dma_start(out=s0["ct_a"], in_=ct_d[0][:, 0:q4])
                nc.scalar.dma_start(out=s0["ct_b"], in_=ct_d[0][:, q4 : 2 * q4])
                nc.gpsimd.dma_start(out=s0["ct_c"], in_=ct_d[0][:, 2 * q4 : NT1])
                nc.sync.dma_start(out=s0["qnb"], in_=qpack_d[0][:, L2 : 2 * L2])
                nc.gpsimd.dma_start(out=s0["c1b"], in_=c1b_d[0])
                nc.scalar.dma_start(out=s1["qpack"], in_=qpack_d[1])
                nc.gpsimd.dma_start(out=s1["aux"], in_=aux_d[1])
                nc.sync.dma_start(out=s1["ct"], in_=ct_d[1])
                nc.gpsimd.dma_start(out=s1["c1b"], in_=c1b_d[1])

            def ct_sl(b, it0, n):
                # contiguous it-range [it0, it0+n) of batch b's transposed ct
                s = S[b]
                if b == 1:
                    return s["ct"][:, it0 : it0 + n]
                q4 = NT1 // 4
                if it0 + n <= q4:
                    return s["ct_a"][:, it0 : it0 + n]
                if it0 >= q4 and it0 + n <= 2 * q4:
                    return s["ct_b"][:, it0 - q4 : it0 - q4 + n]
                assert it0 >= 2 * q4
                return s["ct_c"][:, it0 - 2 * q4 : it0 - 2 * q4 + n]

            def ph_qprep(b):
                s = S[b]
                # qtw = qt*wcq + wc  (shared operand: e2 moving, e1 stationary)
                qtw = work.tile([128, L2], BF16, tag="qtw", name=f"qtw{b}")
                nc.vector.tensor_scalar(
                    out=qtw, in0=s["qt"], scalar1=s["wcq"],
                    scalar2=s["wc"], op0=MULT, op1=ADD,
                )
                s["qtw"] = qtw

            def ph_mask(b):
                # c1m = [mp_i * c | mp_i | 1]  (mask_p + z2 + z1-colsum operand)
                s = S[b]
                c1m = big.tile([128, NT1, 130], BF16, tag="c1m")
                for it in range(NT1):
                    nc.vector.tensor_scalar_mul(
                        c1m[:, it, 0:128], s["c1b"][:, it, :], s["mp"][:, it : it + 1]
                    )
                nc.gpsimd.tensor_copy(
                    c1m[:, :, 128:129].rearrange("p a b -> p (a b)"), s["mp"]
                )
                nc.gpsimd.memset(c1m[:, :, 129:130], 1.0)
                s["c1m"] = c1m

            def e2_strip(b, g):
                # e2[i,j] = exp(dot + cwc_i), unmasked (mp applied via c1m)
                s = S[b]
                if "e2n" not in s:
                    s["e2n"] = big.tile([128, NT1, L2], BF16, tag="e2n",
                                        name=f"e2n{b}")
                st = strips.tile([128, 2, 512], F32, tag="strip")
                for k in range(2):
                    ctt = ct_sl(b, 2 * g + k, 1)
                    nc.tensor.matmul(
                        st[:, k, :], ctt[:, 0, :], s["qtw"],
                        start=True, stop=True,
                    )
                nc.scalar.activation(s["e2n"][:, 2 * g : 2 * g + 2, :], st, EXP)

            def e1_strip(b, jt, h):
                # e1[j,i] = exp(mq_j * (dot + cwc_i)); mq via per-partition
                # ACT scale (j on partitions). masked col -> 1 (uniform)
                s = S[b]
                if "e1" not in s:
                    s["e1"] = big.tile([128, NT2, L1], BF16, tag="e1",
                                       name=f"e1_{b}")
                st = strips.tile([128, 2, 512], F32, tag="strip")
                for k in range(2):
                    m = 2 * h + k
                    nc.tensor.matmul(
                        st[:, k, :], s["qtw"][:, jt * 128 : (jt + 1) * 128],
                        ct_sl(b, 4 * m, 4),
                        start=True, stop=True,
                    )
                nc.scalar.activation(
                    s["e1"][:, jt, 1024 * h : 1024 * (h + 1)],
                    st.rearrange("p a b -> p (a b)"), EXP,
                    scale=s["mq"][:, jt : jt + 1],
                )

            def t_pair(b, jt, g):
                # partial t accumulation (it = 2g, 2g+1): only needs e2n's
                # ACT #g, so these interleave into the e1 chunk stream and
                # fill its strip-rotation waits with real work
                s = S[b]
                if "pst" not in s:
                    s["pst"] = {}
                if jt not in s["pst"]:
                    s["pst"][jt] = small.tile([128, 130], F32, tag="acc",
                                              name=f"pst{b}_{jt}")
                pst = s["pst"][jt]
                for k in range(2):
                    it = 2 * g + k
                    nc.tensor.matmul(
                        pst, s["e2n"][:, it, jt * 128 : (jt + 1) * 128],
                        s["c1m"][:, it, :],
                        start=(it == 0), stop=(it == NT1 - 1),
                    )

            def t_fin(b, jt):
                s = S[b]
                if "rhs_ab" not in s:
                    s["rhs_ab"] = work.tile([128, NT2, 256], BF16, tag="rhs_ab",
                                            name=f"rhs{b}")
                rhs_ab = s["rhs_ab"]
                pst = s["pst"][jt]
                # z1_j = mq_j*(colsum_j - L1) + L1 ; colsum in pst[:,129]
                z1 = work.tile([128, 1], F32, tag="z1")
                nc.vector.scalar_tensor_tensor(
                    out=z1, in0=pst[:, 129:130], scalar=-float(L1),
                    in1=s["mq"][:, jt : jt + 1], op0=ADD, op1=MULT,
                )
                nc.vector.tensor_scalar_add(z1, z1, float(L1))
                rz1 = work.tile([128, 1], F32, tag="rz1")
                nc.vector.reciprocal(rz1, z1)
                rz2 = work.tile([128, 1], F32, tag="rz2")
                nc.vector.reciprocal(rz2, pst[:, 128:129])
                rz12 = work.tile([128, 1], F32, tag="rz12")
                nc.vector.tensor_mul(rz12, rz2, rz1)
                nc.vector.tensor_scalar_mul(
                    rhs_ab[:, jt, 128:256], pst[:, 0:128], rz12
                )
                nc.vector.tensor_scalar_mul(
                    rhs_ab[:, jt, 0:128], s["qnb"][:, jt * 128 : (jt + 1) * 128],
                    rz1,
                )

            def ab_group(b, it0, tg, eng, fill=0, stagger=0):
                s = S[b]
                # fill/stagger: warm mms into a sacrificial acc tile (its
                # banks get reclaimed by the NEXT group's start=True) pad the
                # waits for this group's gating exp ACTs so its matmuls run
                # on a hot PE. Only legal while no OTHER accumulation shares
                # those banks.
                wt = None
                if fill or stagger:
                    wt = small.tile([128, 4, 256], F32, tag="acc", name="warmt")

                def wfill(n):
                    for _ in range(n):
                        nc.tensor.matmul(
                            wt[:, 0, :], warm_b[:, 0:128], warm_b[:, 0:256],
                            start=True, stop=True,
                        )

                wfill(fill)
                psab = small.tile([128, tg, 256], F32, tag="acc", name="psab")
                # groups must accumulate consecutively: a start=True into a
                # bank clears has_written for the whole bank
                for gi in range(tg):
                    it = it0 + gi
                    for jt in range(NT2):
                        nc.tensor.matmul(
                            psab[:, gi, :],
                            s["e1"][:, jt, it * 128 : (it + 1) * 128],
                            s["rhs_ab"][:, jt, :],
                            start=(jt == 0), stop=(jt == NT2 - 1),
                        )
                        if stagger and gi == 0 and jt == NT2 - 2:
                            wfill(stagger)
                o_full = outp.tile([128, 4, 384], BF16, tag="o_sb", name="o_sb")
                o_sb = o_full[:, 0:tg]
                c_sl = s["c1b"][:, it0 : it0 + tg, :]
                # post-exp groups' a-copy rides the scalar engine (idle after
                # its last exp ACT); earlier groups stay on vector
                if b == 1 and it0 >= 8:
                    nc.scalar.copy(o_sb[:, :, 0:128], psab[:, :, 0:128])
                else:
                    nc.vector.tensor_copy(o_sb[:, :, 0:128], psab[:, :, 0:128])
                nc.vector.tensor_tensor(
                    o_sb[:, :, 128:256], c_sl, psab[:, :, 0:128], MULT
                )
                nc.vector.tensor_tensor(
                    o_sb[:, :, 256:384], c_sl, psab[:, :, 128:256], MULT
                )
                odst = out_d[b].rearrange("(p t) m -> p t m", t=NT1)
                if eng is None:
                    # final group: split across two queues for a faster drain
                    nc.sync.dma_start(
                        out=odst[:, it0 : it0 + 2, :], in_=o_sb[:, 0:2]
                    )
                    nc.gpsimd.dma_start(
                        out=odst[:, it0 + 2 : it0 + tg, :], in_=o_sb[:, 2:tg]
                    )
                else:
                    eng.dma_start(out=odst[:, it0 : it0 + tg, :], in_=o_sb)

            def ph_dbg(b):
                if not (dbg and b == 0):
                    return
                s = S[b]
                for name, key in [
                    ("dbg_e1", "e1"), ("dbg_e2n", "e2n"),
                    ("dbg_rhs_ab", "rhs_ab"), ("dbg_qtw", "qtw"),
                    ("dbg_c1m", "c1m"),
                ]:
                    src = s[key]
                    dd = nc.dram_tensor(
                        name, list(src.shape), src.dtype, kind="ExternalOutput"
                    ).ap()
                    nc.sync.dma_start(out=dd, in_=src)

            # v4-style phase alternation (best measured): each engine
            # always has independent cross-batch work adjacent in its queue.
            def e1_phase(b):
                # h-major e1 chunks with the t partial accumulations woven
                # in (their e2n ACT deps are already satisfied, so they fill
                # the strip-rotation waits); each jt's rhs_ab finalizes one
                # chunk after its last partial -> ab can start on the h=0
                # ACTs without waiting for a separate t phase. (Weaving the
                # ab GROUPS into the h=1 chunk stream was tried and badly
                # regressed: a 16-mm group stalls strip production ~3.7us,
                # starving the exp ACT stream.)
                chunks = [(jt, h) for h in range(2) for jt in range(NT2)]
                for k, (jt, h) in enumerate(chunks):
                    e1_strip(b, jt, h)
                    tj = k // 2
                    if k % 2 == 0:
                        for g in range(4):
                            t_pair(b, tj, g)
                    else:
                        for g in range(4, 8):
                            t_pair(b, tj, g)
                        t_fin(b, tj)

            ph_dma_in()
            ph_qprep(0); ph_qprep(1)
            for g in range(8):
                e2_strip(0, g)
            ph_mask(0)
            e1_phase(0)
            ph_mask(1)
            # ab(0) groups interleave with e2(1) strips: the strips are
            # ACT-cadence-limited (strip bufs recycle on ACT drain), so the
            # ab matmuls fill the tensor engine's wait slots
            e2_strip(1, 0); e2_strip(1, 1)
            ab_group(0, 0, 4, nc.sync)
            e2_strip(1, 2); e2_strip(1, 3)
            ab_group(0, 4, 4, nc.sync)
            e2_strip(1, 4); e2_strip(1, 5)
            ab_group(0, 8, 4, nc.sync)
            e2_strip(1, 6); e2_strip(1, 7)
            ab_group(0, 12, 4, nc.sync)
            e1_phase(1)
            # b1 drains on alternating queues so two queues empty in parallel
            ab_group(1, 0, 4, nc.gpsimd)
            ab_group(1, 4, 4, nc.sync)
            ab_group(1, 8, 4, nc.gpsimd)
            ab_group(1, 12, 4, None)
            ph_dbg(0)

    nc.compile()
    return nc


_NC = None


def _get_nc():
    global _NC
    if _NC is None:
        _NC = _build_program()
    return _NC


def _make_in_maps(inputs):
    import ml_dtypes
    bf16 = ml_dtypes.bfloat16

    context, query, w = inputs["context"], inputs["query"], inputs["w"]
    w2 = np.asarray(w).reshape(3, D).astype(np.float32)  # rows: wq, wc, wcq
    mp = np.asarray(inputs["mask_p"]).astype(np.float32)  # (B, L1)
    mq = np.asarray(inputs["mask_q"]).astype(np.float32)  # (B, L2)
    # aux[b] = [mp_t (16) | mq_t (4) | w^T (3)] as [128, 23]
    mp_t = mp.reshape(B, 128, NT1)                        # (B, 128, 16)
    mq_t = mq.reshape(B, 128, NT2)                        # (B, 128, 4)
    wt = np.broadcast_to(w2.T[None], (B, 128, 3))         # (B, 128, 3)
    aux = np.ascontiguousarray(
        np.concatenate([mp_t, mq_t, wt], axis=2), dtype=np.float32
    )
    ctx = np.asarray(context, dtype=np.float32)
    qry = np.asarray(query, dtype=np.float32)
    ct_t = np.ascontiguousarray(
        ctx.reshape(B, 128, NT1, D).transpose(0, 3, 2, 1)
    ).astype(bf16)  # [B, d, it, p]
    c1b = ctx.reshape(B, 128, NT1, D).astype(bf16)        # [B, p, it, d]
    qt_t = np.ascontiguousarray(
        qry.reshape(B, 128, NT2, D).transpose(0, 3, 2, 1)
    ).astype(bf16)  # [B, d, jt, p]
    qnb_t = qry.reshape(B, 128, NT2, D).astype(bf16)      # [B, p, jt, d]
    qpack = np.ascontiguousarray(
        np.concatenate(
            [qt_t.reshape(B, 128, L2), qnb_t.reshape(B, 128, L2)], axis=2
        )
    )  # [B, 128, 1024] bf16
    in_maps = []
    for c in range(NCORES):
        sl = slice(c * BPC, (c + 1) * BPC)
        in_maps.append(
            {
                "aux": aux[sl],
                "qpack": qpack[sl],
                "ct": ct_t[sl],
                "c1b": c1b[sl],
            }
        )
    return in_maps


def kernel(context, query, w, mask_p, mask_q):
    nc = _get_nc()
    in_maps = _make_in_maps(
        {"context": context, "query": query, "w": w, "mask_p": mask_p, "mask_q": mask_q}
    )
    res = bass_utils.run_bass_kernel_spmd(nc, in_maps, core_ids=list(range(NCORES)))
    out = np.empty((B, L1, 4 * D), dtype=np.float32)
    out[:, :, 0:D] = np.asarray(context, dtype=np.float32)  # exact passthrough
    acb = np.concatenate([res.results[c]["out"] for c in range(NCORES)], axis=0)
    out[:, :, D:] = acb.astype(np.float32)
    return out


# revision 48
# speedup vs baseline: 1.2334x; 1.0158x over previous
"""ContextQueryAttention (BiDAF-style) Trainium2 kernel, 8-core data-parallel.

Math (per batch):
  s[i,j]  = wq.q_j + wc.c_i + sum_d c_id * wcq_d * q_jd          (L1 x L2)
  s1      = softmax_i(s * mq_j + (1-mq_j)*NEG)                   (softmax over i)
  s2      = softmax_i(s * mp_i + (1-mp_i)*NEG)
  a       = s1 @ Q                 (L1 x D)
  b       = (s1 @ s2^T) @ C  ==  s1 @ (s2^T @ C)   <- reassociated, no L1xL1
  out     = [C, a, C*a, C*b]                                      (L1 x 4D)

Kernel structure (v11, 62.3us vs 84.1us staged baseline; err 1.1e-4):
 - qwq_j is constant along the softmax axis (i) in both softmaxes -> cancels.
 - wc folded into the shared query-side operand qtw[d,j] = wcq_d*q[j,d] + wc_d
   so BOTH score matmuls produce  psum = dot + cwc_i  directly:
     E1 layout [j part, i free]: stationary qtw tile, moving ct
     E2 layout [i part, j free]: stationary ct tile,  moving qtw
 - mask_q applied via the e1 ACT's per-partition scale operand (j is the
   partition axis there): e1 = exp(mq_j * st); costs +90ns/ACT but kills the
   broadcast-mq tensor + masked qtw copy. Masked col -> exp(0)=1 -> uniform
   1/L1; z1 fixed as z1 = mq_j*(colsum - L1) + L1 from the '1' column.
 - t matmul moving operand c1m[i,:] = [mp_i*c_i | mp_i | 1] gives t, z2 AND
   z1 raw column sums in one accumulation. No reduce ops.
 - whole datapath bf16 except PSUM accumulation; DRAM output bf16 and only
   [a|c*a|c*b] — the context passthrough section is assembled host-side in
   exact f32 (tolerance 2e-2, measured 1.1e-4). DMA 12.4MB -> 6.4MB.
 - scalar engine runs ONLY the 32 exp ACTs (the ~35us serial backbone;
   (N+~260)/1.2ns each, N=1024 bounded by 2-bank strips — widening needs
   PSUM that isn't there) plus the two post-exp a-copies.
 - the whole schedule serves the scalar exp stream: first ACT ~12us (DMA
   bound), zero mid-gaps, T_exp ~47.5us, tail ~14us.
 - t accumulations are split into it-pairs and woven into the e1 chunk
   stream (their e2n ACT deps are long satisfied) so rhs_ab is final before
   the h=1 half and ab needs no separate t phase; ab(0) groups interleave
   with e2(1) strips. Weaving ab GROUPS into e1(1)'s h=1 chunks regresses
   badly (16-mm groups stall strip production -> exp stream gaps).
 - PE clock model (cost_model): ANY idle gap drops the PE to 1.2GHz and it
   needs 3us of CONTINUOUS execution to return to 2.4GHz. Warm mms (memset
   operand, throwaway PSUM tile whose banks the next accumulation's
   start=True reclaims) cover the input-DMA window. Over-filling delays
   in-order work: warm fills between ab tail groups measured WORSE (the
   tail is paced by the psab double-buffer's slow readers, not the clock).
 - per-DMA-queue bandwidth is only ~85-130GB/s: inputs are split into small
   single-reader tiles (ct0 in quarters, qt/qnb separate) spread across the
   sync/scalar/gpsimd queues, ordered by first use; dependency tracking is
   tile-granular, so two DMAs into one tile both gate the first reader.
 - out DMAs: b0 groups on sync while b1 computes; b1 alternates
   gpsimd/sync; the final group splits across both queues.
 - PSUM accumulation groups never interleave within a bank (start=True
   clears has_written for the whole bank); pst(jt0)/pst(jt1) in different
   acc bufs may interleave their accumulations legally.
"""

import numpy as np

import concourse.bass as bass
import concourse.mybir as mybir
import concourse.tile as tile
from concourse import bacc
from concourse import bass_utils

F32 = mybir.dt.float32
BF16 = mybir.dt.bfloat16
EXP = mybir.ActivationFunctionType.Exp
ADD = mybir.AluOpType.add
MULT = mybir.AluOpType.mult

B, L1, L2, D = 16, 2048, 512, 128
NCORES = 8
BPC = B // NCORES          # batches per core
NT1 = L1 // 128            # 16 i-tiles
NT2 = L2 // 128            # 4  j-tiles
NWARM = 10                 # PE pre-warm dummy matmuls (512-wide)


def _build_program(dbg=False):
    nc = bacc.Bacc("TRN2", target_bir_lowering=False, debug=False)

    # aux[b] = [mp_t (16) | mq_t (4) | w^T (3)] as [128, 23] f32, host-packed
    aux_d = nc.dram_tensor("aux", [BPC, 128, NT1 + NT2 + 3], F32,
                           kind="ExternalInput").ap()
    # qpack[b] = [qt (4x128) | qnb (4x128)] bf16, host-packed
    qpack_d = nc.dram_tensor("qpack", [BPC, 128, 2 * NT2 * 128], BF16,
                             kind="ExternalInput").ap()
    ct_d = nc.dram_tensor("ct", [BPC, 128, NT1, 128], BF16,
                          kind="ExternalInput").ap()
    c1b_d = nc.dram_tensor("c1b", [BPC, 128, NT1, 128], BF16,
                           kind="ExternalInput").ap()
    # device output = [a | c*a | c*b]; the context passthrough section is
    # assembled host-side from the (exact f32) input during unshard
    out_d = nc.dram_tensor("out", [BPC, L1, 3 * D], BF16, kind="ExternalOutput").ap()

    with tile.TileContext(nc) as tc:
        with (
            tc.tile_pool(name="const", bufs=1) as const,
            tc.tile_pool(name="big", bufs=2) as big,
            tc.tile_pool(name="work", bufs=2) as work,
            tc.tile_pool(name="outp", bufs=3) as outp,
            tc.tile_pool(name="strips", bufs=2, space="PSUM") as strips,
            tc.tile_pool(name="small", bufs=2, space="PSUM") as small,
        ):
            # PE pre-warm + gap-fill: dummy matmuls on a memset tile. The PE
            # clock gates down on ANY idle gap and takes ~3us of continuous
            # execution to return to 2.4GHz, so warm mms pad the input-DMA
            # window and known dependency waits (ab(1) tail) to keep the
            # busy-streak alive. They write into a THROWAWAY tile from the
            # pool the next real accumulation uses: its first start=True
            # clears the garbage, so no dedicated PSUM bank is needed.
            warm_b = const.tile([128, 512], BF16)
            nc.gpsimd.memset(warm_b, 0.125)

            def warm_fill(pool, shape, n, w):
                wt = pool.tile(shape, F32, tag=("strip" if pool is strips
                                                else "acc"), name="warmt")
                for _ in range(n):
                    nc.tensor.matmul(
                        wt.rearrange("p a b -> p (a b)")[:, 0:w],
                        warm_b[:, 0:128], warm_b[:, 0:w],
                        start=True, stop=True,
                    )

            warm_fill(strips, [128, 2, 512], NWARM, 512)

            S = [dict() for _ in range(BPC)]  # per-batch tile state

            def ph_dma_in():
                # per-queue DMA bandwidth is limited (~100-160GB/s), so the
                # early-critical tensors are small SEPARATE TILES (dependency
                # tracking is tile-granular) spread across the three issuing
                # queues:
                #   sync:   aux0, ct0a, qnb0, ct1
                #   scalar: qt0, ct0b, qpack1     (done by ~7.6us, then ACTs)
                #   gpsimd: c1b0, aux1, c1b1
                for b in range(BPC):
                    s = S[b]
                    s["aux"] = work.tile([128, NT1 + NT2 + 3], F32, tag="aux",
                                         name=f"aux{b}")
                    s["mp"] = s["aux"][:, 0:NT1]
                    s["mq"] = s["aux"][:, NT1 : NT1 + NT2]
                    s["wc"] = s["aux"][:, NT1 + NT2 + 1 : NT1 + NT2 + 2]
                    s["wcq"] = s["aux"][:, NT1 + NT2 + 2 : NT1 + NT2 + 3]
                    s["c1b"] = big.tile([128, NT1, 128], BF16, tag="c1b",
                                        name=f"c1b{b}")
                s0, s1 = S[0], S[1]
                q4 = NT1 // 4
                s0["ct_a"] = big.tile([128, q4, 128], BF16, tag="ct_a",
                                      name="ct0a")
                s0["ct_b"] = big.tile([128, q4, 128], BF16, tag="ct_b",
                                      name="ct0b")
                s0["ct_c"] = big.tile([128, 2 * q4, 128], BF16, tag="ct_c",
                                      name="ct0c")
                s0["qt"] = work.tile([128, L2], BF16, tag="qt", name="qt0")
                s0["qnb"] = work.tile([128, L2], BF16, tag="qnb", name="qnb0")
                s1["qpack"] = work.tile([128, 2 * L2], BF16, tag="qpack",
                                        name="qpack1")
                s1["ct"] = big.tile([128, NT1, 128], BF16, tag="ct",
                                    name="ct1")
                s1["qt"] = s1["qpack"][:, 0:L2]
                s1["qnb"] = s1["qpack"][:, L2 : 2 * L2]
                nc.sync.dma_start(out=s0["aux"], in_=aux_d[0])
                nc.scalar.dma_start(out=s0["qt"], in_=qpack_d[0][:, 0:L2])
                nc.sync.dma_start(out=s0["ct_a"], in_=ct_d[0][:, 0:q4])
                nc.scalar.dma_start(out=s0["ct_b"], in_=ct_d[0][:, q4 : 2 * q4])
                nc.gpsimd.dma_start(out=s0["ct_c"], in_=ct_d[0][:, 2 * q4 : NT1])
                nc.sync.dma_start(out=s0["qnb"], in_=qpack_d[0][:, L2 : 2 * L2])
                nc.gpsimd.dma_start(out=s0["c1b"], in_=c1b_d[0])
                nc.scalar.dma_start(out=s1["qpack"], in_=qpack_d[1])
                nc.gpsimd.dma_start(out=s1["aux"], in_=aux_d[1])
                nc.sync.dma_start(out=s1["ct"], in_=ct_d[1])
                nc.gpsimd.dma_start(out=s1["c1b"], in_=c1b_d[1])

            def ct_sl(b, it0, n):
                # contiguous it-range [it0, it0+n) of batch b's transposed ct
                s = S[b]
                if b == 1:
                    return s["ct"][:, it0 : it0 + n]
                q4 = NT1 // 4
                if it0 + n <= q4:
                    return s["ct_a"][:, it0 : it0 + n]
                if it0 >= q4 and it0 + n <= 2 * q4:
                    return s["ct_b"][:, it0 - q4 : it0 - q4 + n]
                assert it0 >= 2 * q4
                return s["ct_c"][:, it0 - 2 * q4 : it0 - 2 * q4 + n]

            def ph_qprep(b):
                s = S[b]
                # qtw = qt*wcq + wc  (shared operand: e2 moving, e1 stationary)
                qtw = work.tile([128, L2], BF16, tag="qtw", name=f"qtw{b}")
                nc.vector.tensor_scalar(
                    out=qtw, in0=s["qt"], scalar1=s["wcq"],
                    scalar2=s["wc"], op0=MULT, op1=ADD,
                )
                s["qtw"] = qtw

            def ph_mask(b):
                # c1m = [mp_i * c | mp_i | 1]  (mask_p + z2 + z1-colsum operand)
                s = S[b]
                c1m = big.tile([128, NT1, 130], BF16, tag="c1m")
                for it in range(NT1):
                    nc.vector.tensor_scalar_mul(
                        c1m[:, it, 0:128], s["c1b"][:, it, :], s["mp"][:, it : it + 1]
                    )
                nc.gpsimd.tensor_copy(
                    c1m[:, :, 128:129].rearrange("p a b -> p (a b)"), s["mp"]
                )
                nc.gpsimd.memset(c1m[:, :, 129:130], 1.0)
                s["c1m"] = c1m

            def e2_strip(b, g):
                # e2[i,j] = exp(dot + cwc_i), unmasked (mp applied via c1m)
                s = S[b]
                if "e2n" not in s:
                    s["e2n"] = big.tile([128, NT1, L2], BF16, tag="e2n",
                                        name=f"e2n{b}")
                st = strips.tile([128, 2, 512], F32, tag="strip")
                for k in range(2):
                    ctt = ct_sl(b, 2 * g + k, 1)
                    nc.tensor.matmul(
                        st[:, k, :], ctt[:, 0, :], s["qtw"],
                        start=True, stop=True,
                    )
                nc.scalar.activation(s["e2n"][:, 2 * g : 2 * g + 2, :], st, EXP)

            def e1_strip(b, jt, h):
                # e1[j,i] = exp(mq_j * (dot + cwc_i)); mq via per-partition
                # ACT scale (j on partitions). masked col -> 1 (uniform)
                s = S[b]
                if "e1" not in s:
                    s["e1"] = big.tile([128, NT2, L1], BF16, tag="e1",
                                       name=f"e1_{b}")
                st = strips.tile([128, 2, 512], F32, tag="strip")
                for k in range(2):
                    m = 2 * h + k
                    nc.tensor.matmul(
                        st[:, k, :], s["qtw"][:, jt * 128 : (jt + 1) * 128],
                        ct_sl(b, 4 * m, 4),
                        start=True, stop=True,
                    )
                nc.scalar.activation(
                    s["e1"][:, jt, 1024 * h : 1024 * (h + 1)],
                    st.rearrange("p a b -> p (a b)"), EXP,
                    scale=s["mq"][:, jt : jt + 1],
                )

            def t_pair(b, jt, g):
                # partial t accumulation (it = 2g, 2g+1): only needs e2n's
                # ACT #g, so these interleave into the e1 chunk stream and
                # fill its strip-rotation waits with real work
                s = S[b]
                if "pst" not in s:
                    s["pst"] = {}
                if jt not in s["pst"]:
                    s["pst"][jt] = small.tile([128, 130], F32, tag="acc",
                                              name=f"pst{b}_{jt}")
                pst = s["pst"][jt]
                for k in range(2):
                    it = 2 * g + k
                    nc.tensor.matmul(
                        pst, s["e2n"][:, it, jt * 128 : (jt + 1) * 128],
                        s["c1m"][:, it, :],
                        start=(it == 0), stop=(it == NT1 - 1),
                    )

            def t_fin(b, jt):
                s = S[b]
                if "rhs_ab" not in s:
                    s["rhs_ab"] = work.tile([128, NT2, 256], BF16, tag="rhs_ab",
                                            name=f"rhs{b}")
                rhs_ab = s["rhs_ab"]
                pst = s["pst"][jt]
                # z1_j = mq_j*(colsum_j - L1) + L1 ; colsum in pst[:,129]
                z1 = work.tile([128, 1], F32, tag="z1")
                nc.vector.scalar_tensor_tensor(
                    out=z1, in0=pst[:, 129:130], scalar=-float(L1),
                    in1=s["mq"][:, jt : jt + 1], op0=ADD, op1=MULT,
                )
                nc.vector.tensor_scalar_add(z1, z1, float(L1))
                rz1 = work.tile([128, 1], F32, tag="rz1")
                nc.vector.reciprocal(rz1, z1)
                rz2 = work.tile([128, 1], F32, tag="rz2")
                nc.vector.reciprocal(rz2, pst[:, 128:129])
                rz12 = work.tile([128, 1], F32, tag="rz12")
                nc.vector.tensor_mul(rz12, rz2, rz1)
                nc.vector.tensor_scalar_mul(
                    rhs_ab[:, jt, 128:256], pst[:, 0:128], rz12
                )
                nc.vector.tensor_scalar_mul(
                    rhs_ab[:, jt, 0:128], s["qnb"][:, jt * 128 : (jt + 1) * 128],
                    rz1,
                )

            def ab_group(b, it0, tg, eng, fill=0, stagger=0, ppool=None):
                s = S[b]
                # fill/stagger: warm mms into a sacrificial acc tile (its
                # banks get reclaimed by the NEXT group's start=True) pad the
                # waits for this group's gating exp ACTs so its matmuls run
                # on a hot PE. Only legal while no OTHER accumulation shares
                # those banks.
                wt = None
                if fill or stagger:
                    wt = small.tile([128, 4, 256], F32, tag="acc", name="warmt")

                def wfill(n):
                    for _ in range(n):
                        nc.tensor.matmul(
                            wt[:, 0, :], warm_b[:, 0:128], warm_b[:, 0:256],
                            start=True, stop=True,
                        )

                wfill(fill)
                # terminal groups borrow the strips pool (idle once the last
                # e1 strip drains; same 2-bank footprint) so their matmuls
                # don't wait on the acc slots' slow o_sb readers
                pool = ppool if ppool is not None else small
                psab = pool.tile([128, tg, 256], F32,
                                 tag=("strip" if pool is strips else "acc"),
                                 name="psab")
                # groups must accumulate consecutively: a start=True into a
                # bank clears has_written for the whole bank
                for gi in range(tg):
                    it = it0 + gi
                    for jt in range(NT2):
                        nc.tensor.matmul(
                            psab[:, gi, :],
                            s["e1"][:, jt, it * 128 : (it + 1) * 128],
                            s["rhs_ab"][:, jt, :],
                            start=(jt == 0), stop=(jt == NT2 - 1),
                        )
                        if stagger and gi == 0 and jt == NT2 - 2:
                            wfill(stagger)
                o_full = outp.tile([128, 4, 384], BF16, tag="o_sb", name="o_sb")
                o_sb = o_full[:, 0:tg]
                c_sl = s["c1b"][:, it0 : it0 + tg, :]
                # b1's a-copies ride the scalar engine: program order puts
                # them after every exp ACT, and scalar is idle from then on
                if b == 1:
                    nc.scalar.copy(o_sb[:, :, 0:128], psab[:, :, 0:128])
                else:
                    nc.vector.tensor_copy(o_sb[:, :, 0:128], psab[:, :, 0:128])
                nc.vector.tensor_tensor(
                    o_sb[:, :, 128:256], c_sl, psab[:, :, 0:128], MULT
                )
                nc.vector.tensor_tensor(
                    o_sb[:, :, 256:384], c_sl, psab[:, :, 128:256], MULT
                )
                odst = out_d[b].rearrange("(p t) m -> p t m", t=NT1)
                if eng is None:
                    # final group: split across two queues for a faster drain
                    nc.sync.dma_start(
                        out=odst[:, it0 : it0 + 2, :], in_=o_sb[:, 0:2]
                    )
                    nc.gpsimd.dma_start(
                        out=odst[:, it0 + 2 : it0 + tg, :], in_=o_sb[:, 2:tg]
                    )
                else:
                    eng.dma_start(out=odst[:, it0 : it0 + tg, :], in_=o_sb)

            def ph_dbg(b):
                if not (dbg and b == 0):
                    return
                s = S[b]
                for name, key in [
                    ("dbg_e1", "e1"), ("dbg_e2n", "e2n"),
                    ("dbg_rhs_ab", "rhs_ab"), ("dbg_qtw", "qtw"),
                    ("dbg_c1m", "c1m"),
                ]:
                    src = s[key]
                    dd = nc.dram_tensor(
                        name, list(src.shape), src.dtype, kind="ExternalOutput"
                    ).ap()
                    nc.sync.dma_start(out=dd, in_=src)

            # v4-style phase alternation (best measured): each engine
            # always has independent cross-batch work adjacent in its queue.
            def e1_phase(b):
                # h-major e1 chunks with the t partial accumulations woven
                # in (their e2n ACT deps are already satisfied, so they fill
                # the strip-rotation waits); each jt's rhs_ab finalizes one
                # chunk after its last partial -> ab can start on the h=0
                # ACTs without waiting for a separate t phase. (Weaving the
                # ab GROUPS into the h=1 chunk stream was tried and badly
                # regressed: a 16-mm group stalls strip production ~3.7us,
                # starving the exp ACT stream.)
                chunks = [(jt, h) for h in range(2) for jt in range(NT2)]
                for k, (jt, h) in enumerate(chunks):
                    e1_strip(b, jt, h)
                    tj = k // 2
                    if k % 2 == 0:
                        for g in range(4):
                            t_pair(b, tj, g)
                    else:
                        for g in range(4, 8):
                            t_pair(b, tj, g)
                        t_fin(b, tj)

            ph_dma_in()
            ph_qprep(0); ph_qprep(1)
            for g in range(8):
                e2_strip(0, g)
            ph_mask(0)
            e1_phase(0)
            ph_mask(1)
            # ab(0) groups interleave with e2(1) strips: the strips are
            # ACT-cadence-limited (strip bufs recycle on ACT drain), so the
            # ab matmuls fill the tensor engine's wait slots
            e2_strip(1, 0); e2_strip(1, 1)
            ab_group(0, 0, 4, nc.sync)
            e2_strip(1, 2); e2_strip(1, 3)
            ab_group(0, 4, 4, nc.sync)
            e2_strip(1, 4); e2_strip(1, 5)
            ab_group(0, 8, 4, nc.sync)
            e2_strip(1, 6); e2_strip(1, 7)
            ab_group(0, 12, 4, nc.sync)
            e1_phase(1)
            # b1 drains on alternating queues so two queues empty in parallel
            ab_group(1, 0, 4, nc.gpsimd)
            ab_group(1, 4, 4, nc.sync)
            ab_group(1, 8, 4, nc.gpsimd, ppool=strips)
            ab_group(1, 12, 4, None, ppool=strips)
            ph_dbg(0)

    nc.compile()
    return nc


_NC = None


def _get_nc():
    global _NC
    if _NC is None:
        _NC = _build_program()
    return _NC


def _make_in_maps(inputs):
    import ml_dtypes
    bf16 = ml_dtypes.bfloat16

    context, query, w = inputs["context"], inputs["query"], inputs["w"]
    w2 = np.asarray(w).reshape(3, D).astype(np.float32)  # rows: wq, wc, wcq
    mp = np.asarray(inputs["mask_p"]).astype(np.float32)  # (B, L1)
    mq = np.asarray(inputs["mask_q"]).astype(np.float32)  # (B, L2)
    # aux[b] = [mp_t (16) | mq_t (4) | w^T (3)] as [128, 23]
    mp_t = mp.reshape(B, 128, NT1)                        # (B, 128, 16)
    mq_t = mq.reshape(B, 128, NT2)                        # (B, 128, 4)
    wt = np.broadcast_to(w2.T[None], (B, 128, 3))         # (B, 128, 3)
    aux = np.ascontiguousarray(
        np.concatenate([mp_t, mq_t, wt], axis=2), dtype=np.float32
    )
    ctx = np.asarray(context, dtype=np.float32)
    qry = np.asarray(query, dtype=np.float32)
    ct_t = np.ascontiguousarray(
        ctx.reshape(B, 128, NT1, D).transpose(0, 3, 2, 1)
    ).astype(bf16)  # [B, d, it, p]
    c1b = ctx.reshape(B, 128, NT1, D).astype(bf16)        # [B, p, it, d]
    qt_t = np.ascontiguousarray(
        qry.reshape(B, 128, NT2, D).transpose(0, 3, 2, 1)
    ).astype(bf16)  # [B, d, jt, p]
    qnb_t = qry.reshape(B, 128, NT2, D).astype(bf16)      # [B, p, jt, d]
    qpack = np.ascontiguousarray(
        np.concatenate(
            [qt_t.reshape(B, 128, L2), qnb_t.reshape(B, 128, L2)], axis=2
        )
    )  # [B, 128, 1024] bf16
    in_maps = []
    for c in range(NCORES):
        sl = slice(c * BPC, (c + 1) * BPC)
        in_maps.append(
            {
                "aux": aux[sl],
                "qpack": qpack[sl],
                "ct": ct_t[sl],
                "c1b": c1b[sl],
            }
        )
    return in_maps


def kernel(context, query, w, mask_p, mask_q):
    nc = _get_nc()
    in_maps = _make_in_maps(
        {"context": context, "query": query, "w": w, "mask_p": mask_p, "mask_q": mask_q}
    )
    res = bass_utils.run_bass_kernel_spmd(nc, in_maps, core_ids=list(range(NCORES)))
    out = np.empty((B, L1, 4 * D), dtype=np.float32)
    out[:, :, 0:D] = np.asarray(context, dtype=np.float32)  # exact passthrough
    acb = np.concatenate([res.results[c]["out"] for c in range(NCORES)], axis=0)
    out[:, :, D:] = acb.astype(np.float32)
    return out
